# revision 18
# baseline (speedup 1.0000x reference)
"""Trainium2 Bass kernel for Masked_Actor_Net_PNAConv (3x PNAConv + gated masked softmax head).

Sharding: data-parallel by graph across 8 NeuronCores (8 graphs / 2048 nodes /
16384 edges per core). Weights replicated. BatchNorm batch stats are
all-reduced across cores (one [128, 2*Fo] f32 AllReduce per conv layer).

Device-side structure (per core, per layer):
  - h kept feature-major in SBUF arena hT [128, 4, 2048] bf16
  - A = h @ Wm_src computed node-major on PE -> a_sb (SBUF only, no DRAM trip)
  - per-edge messages materialized as PE matmuls: for each 512-edge tile,
    msg^T[f, e] accumulates 3 matmuls in PSUM: one-hot(src lo chunk) @ A_lo +
    one-hot(src hi chunk) @ A_hi + Wmc^T @ es^T  (no DMA gather, no gpsimd)
  - edge columns ordered j-innermost (node-major, 8 edges contiguous):
    max agg = vector pool_max over window 8 straight from PSUM;
    sumsq agg = scalar Square eviction + vector pool_avg (gives E[msg^2])
  - mean agg via mean-adjacency (counts/8) matmuls + es-mean term on PE
  - U matmuls run full-width (512-node tiles across graphs); BatchNorm folded
    into the mixing Linear after the stats AllReduce (as before)
"""
import sys
sys.path.insert(0, '/opt/trn_rl_repo')
import contextlib
import numpy as np
import ml_dtypes

import concourse.bacc as bacc
import concourse.mybir as mybir
import concourse.bass_isa as bass_isa
from concourse import tile
from concourse.bass_utils import run_bass_kernel_spmd

BF = mybir.dt.bfloat16
F32 = mybir.dt.float32
U8 = mybir.dt.uint8
AL = mybir.AluOpType
AF = mybir.ActivationFunctionType
AX = mybir.AxisListType

B, NN, DEG = 64, 256, 8
N, E = B * NN, B * NN * DEG
IN_N, IN_E = 128, 16
TP = 192
H1 = 384
NCORES = 8
G = B // NCORES        # 8 graphs per core
NC = G * NN            # 2048 nodes per core
EC = NC * DEG          # 16384 edges per core

CIN = [IN_N, H1 + 32, H1]                 # 128, 416, 384
COUT = [H1, H1, TP]                       # 384, 384, 192
NF = [(c + 127) // 128 for c in CIN]      # 1, 4, 3
CINP = [128 * f for f in NF]              # 128, 512, 384
NFO = [(c + 127) // 128 for c in COUT]    # 3, 3, 2
CSZ = [[min(128, CIN[k] - 128 * i) for i in range(NF[k])] for k in range(3)]
MSZ = [[min(128, COUT[k] - 128 * i) for i in range(NFO[k])] for k in range(3)]
NSEC = [4, 13, 12]  # U weight sections per layer (L1: 4x3 full + 1 packed)

_BUILT = {}


def _bf(x):
    return np.ascontiguousarray(np.asarray(x, np.float32).astype(ml_dtypes.bfloat16))


def _f32(x):
    return np.ascontiguousarray(np.asarray(x, np.float32))


# ---------------------------------------------------------------------------
# device kernel (SPMD, identical program on all 8 cores)
# ---------------------------------------------------------------------------

def build_nc():
    nc = bacc.Bacc(None, target_bir_lowering=False, debug=True, dynamic_dma_scratch_size=32768)

    def par(name, shape, dt, out=False):
        return nc.declare_dram_parameter(name, list(shape), dt, isOutput=out)

    p_nsT = par("nsT", [128, 2048], BF)
    p_dmT = par("dmT", [128, 2 * 2048], BF)
    p_esm = par("esm", [16, 2048], BF)
    p_cg = [par(f"cg{k}", [128, G * 2 * NF[k] * 1024], BF) for k in range(3)]
    p_oh = par("oh", [128, G * 2 * 2048], BF)
    p_mam = par("mam", [128, G * 2 * 256], BF)
    p_mask = par("mask", [128, 16 * 192], BF)
    p_wma = [par(f"wma{k}", [128, NF[k] * CINP[k]], BF) for k in range(3)]
    p_wmc = [par(f"wmc{k}", [16, CINP[k]], BF) for k in range(3)]
    p_wu = [par(f"wu{k}", [128, NSEC[k] * COUT[k]], BF) for k in range(3)]
    p_wx = [par(f"wx{k}", [128, NFO[k] * COUT[k]], BF) for k in range(3)]
    p_gam = [par(f"gam{k}", [128, NFO[k]], F32) for k in range(3)]
    p_bh = [par(f"bh{k}", [128, NFO[k]], F32) for k in range(2)]
    p_w12 = par("w12", [128, 2 * 32], BF)
    p_b12 = par("b12", [32, 1], F32)
    p_w3 = par("w3", [128, 2 * 64], BF)
    p_b3 = par("b3", [64, 1], F32)
    p_w4 = par("w4", [64, 256], BF)
    p_b4 = par("b4", [128, 2], F32)
    p_out = par("out", [128, 16 * 192], F32, out=True)

    with tile.TileContext(nc) as tc:
        with contextlib.ExitStack() as ctx:
            stat = ctx.enter_context(tc.tile_pool(name="stat", bufs=1))
            ohp = ctx.enter_context(tc.tile_pool(name="ohp", bufs=2))    # one-hot per graph
            cgp = ctx.enter_context(tc.tile_pool(name="cgp", bufs=2))    # host C slices
            msgp = ctx.enter_context(tc.tile_pool(name="msgp", bufs=2))  # evicted msg tiles
            sqp = ctx.enter_context(tc.tile_pool(name="sqp", bufs=2))    # square scratch
            wupool = ctx.enter_context(tc.tile_pool(name="wupool", bufs=1))
            dpool = ctx.enter_context(tc.tile_pool(name="dpool", bufs=1, space="DRAM"))
            psM = ctx.enter_context(tc.tile_pool(name="psM", bufs=2, space="PSUM"))
            psU = ctx.enter_context(tc.tile_pool(name="psU", bufs=2, space="PSUM"))
            psW = ctx.enter_context(tc.tile_pool(name="psW", bufs=2, space="PSUM"))

            def load(shape, dt, src, tag, pool=None):
                t = (pool or stat).tile(list(shape), dt, tag=tag, name=tag)
                nc.sync.dma_start(t[:], src[:])
                return t

            hT = stat.tile([128, 4, 2048], BF, tag="hT", name="hT")
            nc.sync.dma_start(hT[:, 0, :], p_nsT[:])
            uT = stat.tile([128, 3, 2048], BF, tag="uT")
            pmax = stat.tile([128, 4, 2048], BF, tag="pmax")
            pmean = stat.tile([128, 4, 2048], BF, tag="pmean")
            pstd = stat.tile([128, 4, 2048], BF, tag="pstd")
            a_sb = stat.tile([128, 16, 512], BF, tag="a_sb")
            qmean = stat.tile([128, 4, 256], F32, tag="qmean")
            vtmp = stat.tile([128, 4, 256], F32, tag="vtmp")
            esm = load([16, 2048], BF, p_esm, "esm")
            mam = load([128, G, 2, 256], BF, p_mam, "mam")
            packX = None  # allocated at use; shares the dmT slot

            wmc = [load([16, CINP[k]], BF, p_wmc[k], f"wmc{k}s") for k in range(3)]
            wx = [load([128, NFO[k], COUT[k]], BF, p_wx[k], f"wx{k}s") for k in range(3)]
            gam = [load([128, NFO[k]], F32, p_gam[k], f"gam{k}s") for k in range(3)]
            bh = [load([128, NFO[k]], F32, p_bh[k], f"bh{k}s") for k in range(2)]
            w12 = load([128, 2, 32], BF, p_w12, "w12")
            b12 = load([32, 1], F32, p_b12, "b12")
            w3 = load([128, 2, 64], BF, p_w3, "w3")
            b3 = load([64, 1], F32, p_b3, "b3")
            w4 = load([64, 256], BF, p_w4, "w4")
            b4 = load([128, 2], F32, p_b4, "b4")
            wxs = stat.tile([128, 3, 384], BF, tag="wxs")
            cc_in = [dpool.tile([128, 2 * NFO[k]], F32, tag=f"ccin{k}", name=f"ccin{k}") for k in range(3)]
            cc_out = [dpool.tile([128, 2 * NFO[k]], F32, tag=f"ccout{k}", name=f"ccout{k}") for k in range(3)]

            # ---- d2 = dm @ (W1 @ W2) + b12 -> hT chunk 3 rows 0:32 -----------
            dmT = stat.tile([128, 2, 2048], BF, tag="packX")  # slot shared with packX
            nc.sync.dma_start(dmT[:].rearrange("p c n -> p (c n)"), p_dmT[:])
            for n4 in range(4):
                ps = psW.tile([128, 512], F32, tag="psW")
                for kc in range(2):
                    nc.tensor.matmul(ps[0:32, :], w12[:, kc, :],
                                     dmT[:, kc, 512 * n4:512 * (n4 + 1)],
                                     start=(kc == 0), stop=(kc == 1))
                nc.scalar.activation(hT[0:32, 3, 512 * n4:512 * (n4 + 1)], ps[0:32, :],
                                     AF.Identity, bias=b12[:, 0:1])

            h3 = stat.tile([128, 16, 192], BF, tag="h3")
            c30 = stat.tile([128, 1], F32, tag="c30")
            nc.vector.memset(c30[:], 1e-30)
            c5 = stat.tile([128, 1], F32, tag="c5")
            nc.vector.memset(c5[:], 1e-5)
            uaccS = stat.tile([128, 3, 4], F32, tag="uaccS")
            uaccQ = stat.tile([128, 3, 4], F32, tag="uaccQ")

            # ---- conv layers -------------------------------------------------
            for k in range(3):
                F = NF[k]
                cinp, cout, Fo = CINP[k], COUT[k], NFO[k]
                csz, msz = CSZ[k], MSZ[k]

                wu_k = load([128, NSEC[k], cout], BF, p_wu[k], "wu_k", pool=wupool)
                wma_k = load([128, F, cinp], BF, p_wma[k], "wma_k", pool=wupool)
                nc.vector.memset(uaccS[:], 0.0)
                nc.vector.memset(uaccQ[:], 0.0)
                if k == 2:
                    nc.vector.memset(uT[64:128, 1, :], 0.0)

                # A = h @ Wma (node-major) -> a_sb
                for t in range(16):
                    ps = psU.tile([128, 512], F32, tag="psU")
                    for ki in range(F):
                        nc.tensor.matmul(ps[:, 0:cinp],
                                         hT[0:csz[ki], ki, 128 * t:128 * (t + 1)],
                                         wma_k[0:csz[ki], ki, :],
                                         start=(ki == 0), stop=(ki == F - 1))
                    nc.scalar.activation(a_sb[:, t, 0:cinp], ps[:, 0:cinp], AF.Copy, bias=0.0)

                # aggregation phase, per graph
                for g in range(G):
                    ohg = ohp.tile([128, 2, 2048], BF, tag="ohg")
                    nc.sync.dma_start(ohg[:].rearrange("p c e -> p (c e)"),
                                      p_oh[:, g * 4096:(g + 1) * 4096])

                    for f in range(F):
                        # mean aggregation: A^T @ madj/8 + Wmc^T @ es_mean
                        pm = psU.tile([128, 512], F32, tag="psU")
                        nc.tensor.matmul(pm[:, 0:256], a_sb[:, 2 * g, 128 * f:128 * (f + 1)],
                                         mam[:, g, 0, :], start=True, stop=False)
                        nc.tensor.matmul(pm[:, 0:256], a_sb[:, 2 * g + 1, 128 * f:128 * (f + 1)],
                                         mam[:, g, 1, :], start=False, stop=False)
                        nc.tensor.matmul(pm[:, 0:256], wmc[k][0:16, 128 * f:128 * (f + 1)],
                                         esm[0:16, 256 * g:256 * (g + 1)],
                                         start=False, stop=True)
                        nc.scalar.activation(pmean[:, f, 256 * g:256 * (g + 1)], pm[:, 0:256],
                                             AF.Copy, bias=0.0)

                    # messages: PE does the two one-hot gathers only; the es
                    # contribution C = es @ Wmc is host-computed and added on
                    # the (otherwise idle) gpsimd after a bf16 eviction.
                    for h2 in range(2):
                        cgh = cgp.tile([128, F, 1024], BF, tag="cgh")
                        nc.sync.dma_start(
                            cgh[:].rearrange("p f e -> p (f e)"),
                            p_cg[k][:, (g * 2 + h2) * F * 1024:(g * 2 + h2 + 1) * F * 1024])
                        for f in range(F):
                            pmsg = psM.tile([128, 2, 512], F32, tag="psM")
                            for e2 in range(2):
                                c0 = 1024 * h2 + 512 * e2
                                nc.tensor.matmul(pmsg[:, e2, :],
                                                 a_sb[:, 2 * g, 128 * f:128 * (f + 1)],
                                                 ohg[:, 0, c0:c0 + 512], start=True, stop=False)
                                nc.tensor.matmul(pmsg[:, e2, :],
                                                 a_sb[:, 2 * g + 1, 128 * f:128 * (f + 1)],
                                                 ohg[:, 1, c0:c0 + 512], start=False, stop=True)
                            msgb = msgp.tile([128, 1024], BF, tag="msgb")
                            nc.scalar.activation(msgb[:], pmsg[:].rearrange("p a e -> p (a e)"),
                                                 AF.Copy, bias=0.0)
                            nc.gpsimd.tensor_tensor(msgb[:], msgb[:], cgh[:, f, :], AL.add)
                            # max over the 8 edges of each node (j innermost)
                            nc.vector.tensor_reduce(
                                pmax[:, f, 256 * g + 128 * h2:256 * g + 128 * (h2 + 1)],
                                msgb[:].rearrange("p (n j) -> p n j", j=8), AX.X, AL.max)
                            sq = sqp.tile([128, 1024], BF, tag="sq")
                            if k == 1:
                                nc.gpsimd.tensor_tensor(sq[:], msgb[:], msgb[:], AL.mult)
                            else:
                                nc.scalar.activation(sq[:], msgb[:], AF.Square)
                            nc.vector.tensor_reduce(
                                qmean[:, f, 128 * h2:128 * (h2 + 1)],
                                sq[:].rearrange("p (n j) -> p n j", j=8), AX.X, AL.add)

                    # std = sqrt(relu(Q/8 - mean^2) + 1e-30)
                    nc.scalar.activation(qmean[:, 0:F, :], qmean[:, 0:F, :],
                                         AF.Copy, bias=0.0, scale=0.125)
                    nc.vector.tensor_tensor(vtmp[:, 0:F, :],
                                            pmean[:, 0:F, 256 * g:256 * (g + 1)],
                                            pmean[:, 0:F, 256 * g:256 * (g + 1)], AL.mult)
                    nc.vector.tensor_tensor(qmean[:, 0:F, :], qmean[:, 0:F, :],
                                            vtmp[:, 0:F, :], AL.subtract)
                    nc.scalar.activation(qmean[:, 0:F, :], qmean[:, 0:F, :], AF.Relu)
                    nc.scalar.activation(pstd[:, 0:F, 256 * g:256 * (g + 1)],
                                         qmean[:, 0:F, :], AF.Sqrt, bias=c30[:, 0:1])

                # U matmuls: X = [h | mean | max | std], folded weights, 512-wide.
                # For L1 the four ragged 32-row f=3 chunks are packed into one
                # K=128 section (rows: h-d2 | mean | max | std), built via DMA.
                xs = [None, pmean, pmax, pstd]
                if k == 1:
                    packX = stat.tile([128, 2048], BF, tag="packX")
                    nc.sync.dma_start(packX[0:32, :], hT[0:32, 3, :])
                    nc.sync.dma_start(packX[32:64, :], pmean[0:32, 3, :])
                    nc.sync.dma_start(packX[64:96, :], pmax[0:32, 3, :])
                    nc.sync.dma_start(packX[96:128, :], pstd[0:32, 3, :])
                    nfull = 3
                else:
                    nfull = F
                nsec = 4 * nfull + (1 if k == 1 else 0)
                for mo in range(Fo):
                    mi = msz[mo]
                    for n4 in range(4):
                        ps = psU.tile([128, 512], F32, tag="psU")
                        i = 0
                        for sect in range(4):
                            for f in range(nfull):
                                if sect == 0:
                                    rhs = hT[0:csz[f], f, 512 * n4:512 * (n4 + 1)]
                                else:
                                    rhs = xs[sect][0:csz[f], f, 512 * n4:512 * (n4 + 1)]
                                nc.tensor.matmul(
                                    ps[0:mi, :],
                                    wu_k[0:csz[f], sect * nfull + f, 128 * mo:128 * mo + mi],
                                    rhs, start=(i == 0), stop=(i == nsec - 1))
                                i += 1
                        if k == 1:
                            nc.tensor.matmul(
                                ps[0:mi, :],
                                wu_k[:, 4 * nfull, 128 * mo:128 * mo + mi],
                                packX[:, 512 * n4:512 * (n4 + 1)],
                                start=False, stop=True)
                            i += 1
                        nc.scalar.activation(uT[0:mi, mo, 512 * n4:512 * (n4 + 1)], ps[0:mi, :],
                                             AF.Copy, bias=0.0,
                                             accum_out=uaccS[0:mi, mo, n4:n4 + 1])
                        usq = sqp.tile([128, 1024], BF, tag="sq")
                        nc.scalar.activation(usq[0:mi, 0:512], uT[0:mi, mo, 512 * n4:512 * (n4 + 1)],
                                             AF.Square, accum_out=uaccQ[0:mi, mo, n4:n4 + 1])

                # ---- BN stats all-reduce, fold into mixing ----
                ccs = stat.tile([128, 6], F32, tag="ccs")
                nc.vector.tensor_reduce(ccs[:, 0:Fo], uaccS[:, 0:Fo, :], AX.X, AL.add)
                nc.vector.tensor_reduce(ccs[:, Fo:2 * Fo], uaccQ[:, 0:Fo, :], AX.X, AL.add)
                nc.sync.dma_start(cc_in[k][:], ccs[:, 0:2 * Fo])
                import os as _os
                _rg = [[i] for i in range(NCORES)] if _os.environ.get("KERN_NO_CC") else [list(range(NCORES))]
                nc.gpsimd.collective_compute(
                    "AllReduce", AL.add, replica_groups=_rg,
                    ins=[cc_in[k].opt()], outs=[cc_out[k].opt()])
                ccr = stat.tile([128, 6], F32, tag="ccr")
                nc.sync.dma_start(ccr[:, 0:2 * Fo], cc_out[k][:])
                mu = stat.tile([128, 3], F32, tag="mu")
                sc = stat.tile([128, 3], F32, tag="sc")
                mu2 = stat.tile([128, 3], F32, tag="mu2")
                nc.scalar.activation(mu[:, 0:Fo], ccr[:, 0:Fo], AF.Copy, bias=0.0, scale=1.0 / N)
                nc.scalar.activation(sc[:, 0:Fo], ccr[:, Fo:2 * Fo], AF.Copy, bias=0.0, scale=1.0 / N)
                nc.vector.tensor_tensor(mu2[:, 0:Fo], mu[:, 0:Fo], mu[:, 0:Fo], AL.mult)
                nc.vector.tensor_tensor(sc[:, 0:Fo], sc[:, 0:Fo], mu2[:, 0:Fo], AL.subtract)
                nc.scalar.activation(sc[:, 0:Fo], sc[:, 0:Fo], AF.Sqrt, bias=c5[:, 0:1])
                nc.vector.reciprocal(sc[:, 0:Fo], sc[:, 0:Fo])
                nc.vector.tensor_tensor(sc[:, 0:Fo], sc[:, 0:Fo], gam[k][:, 0:Fo], AL.mult)
                for mo in range(Fo):
                    mi = msz[mo]
                    nc.vector.tensor_scalar(uT[0:mi, mo, :], uT[0:mi, mo, :],
                                            mu[0:mi, mo:mo + 1], None, AL.subtract)
                    nc.vector.tensor_scalar(wxs[:, mo, 0:cout], wx[k][:, mo, 0:cout],
                                            sc[:, mo:mo + 1], None, AL.mult)
                if k == 2:
                    nc.vector.memset(uT[64:65, 1, :], 1.0)
                # mixing matmul (+ BN shift via bias / ones-row), relu(leaky) = relu
                if k < 2:
                    for mo in range(Fo):
                        for n4 in range(4):
                            ps = psW.tile([128, 512], F32, tag="psW")
                            for mk in range(Fo):
                                nc.tensor.matmul(ps[:, :],
                                                 wxs[0:msz[mk], mk, 128 * mo:128 * (mo + 1)],
                                                 uT[0:msz[mk], mk, 512 * n4:512 * (n4 + 1)],
                                                 start=(mk == 0), stop=(mk == Fo - 1))
                            nc.scalar.activation(hT[:, mo, 512 * n4:512 * (n4 + 1)], ps[:, :],
                                                 AF.Relu, bias=bh[k][:, mo:mo + 1])
                else:
                    for t in range(16):
                        ps = psW.tile([128, 512], F32, tag="psW")
                        nc.tensor.matmul(ps[:, 0:192], uT[0:128, 0, 128 * t:128 * (t + 1)],
                                         wxs[0:128, 0, 0:192], start=True, stop=False)
                        nc.tensor.matmul(ps[:, 0:192], uT[0:65, 1, 128 * t:128 * (t + 1)],
                                         wxs[0:65, 1, 0:192], start=False, stop=True)
                        nc.scalar.activation(h3[:, t, :], ps[:, 0:192], AF.Lrelu, alpha=0.01)

            # ---- head --------------------------------------------------------
            mask = load([128, 16, 192], BF, p_mask, "wu_k", pool=wupool)  # wu slot is dead now
            nmx = stat.tile([128, 16], BF, tag="nmx")
            nc.vector.tensor_reduce(nmx[:], h3[:], AX.X, AL.max)
            ps3 = psW.tile([128, 512], F32, tag="psW")
            nc.tensor.matmul(ps3[0:64, 0:8], w3[:, 0, :], nmx[:, 0::2], start=True, stop=False)
            nc.tensor.matmul(ps3[0:64, 0:8], w3[:, 1, :], nmx[:, 1::2], start=False, stop=True)
            r3 = stat.tile([64, 8], BF, tag="r3")
            nc.scalar.activation(r3[:], ps3[0:64, 0:8], AF.Relu, bias=b3[:, 0:1])
            gn = stat.tile([128, 16], F32, tag="gn")
            for half in range(2):
                ps4 = psW.tile([128, 512], F32, tag="psW")
                nc.tensor.matmul(ps4[:, 0:8], w4[0:64, 128 * half:128 * (half + 1)], r3[:],
                                 start=True, stop=True)
                nc.scalar.activation(gn[:, half::2], ps4[:, 0:8], AF.Sigmoid,
                                     bias=b4[:, half:half + 1])
            feat = stat.tile([128, 16, 192], F32, tag="pmax")   # reuse agg slots at head time
            for c in range(16):
                nc.vector.tensor_scalar(feat[:, c, :], h3[:, c, :], gn[:, c:c + 1], None, AL.mult)
            fm = stat.tile([128, 16, 192], F32, tag="pmean")
            # fm = (feat + 1e5) * mask01: masked lanes -> 0, ~1e5 below valid
            nc.vector.scalar_tensor_tensor(fm[:], feat[:], 1e5, mask[:],
                                           AL.add, AL.mult)
            gmax = stat.tile([128, 8], F32, tag="gmax")
            gmaxr = stat.tile([128, 8], F32, tag="gmaxr")
            nc.vector.tensor_reduce(gmax[:], fm[:].rearrange("p (g x) t -> p g (x t)", g=8), AX.X, AL.max)
            nc.gpsimd.partition_all_reduce(gmaxr[:], gmax[:], 128, bass_isa.ReduceOp.max)
            for g in range(8):
                nc.vector.tensor_scalar(fm[:, 2 * g:2 * (g + 1), :], fm[:, 2 * g:2 * (g + 1), :],
                                        gmaxr[:, g:g + 1], None, AL.subtract)
            nc.scalar.activation(fm[:], fm[:], AF.Exp)
            gsum = stat.tile([128, 8], F32, tag="gsum")
            gsumr = stat.tile([128, 8], F32, tag="gsumr")
            nc.vector.tensor_reduce(gsum[:], fm[:].rearrange("p (g x) t -> p g (x t)", g=8), AX.X, AL.add)
            nc.gpsimd.partition_all_reduce(gsumr[:], gsum[:], 128, bass_isa.ReduceOp.add)
            nc.vector.reciprocal(gsumr[:], gsumr[:])
            osb = stat.tile([128, 16, 192], F32, tag="pstd")
            for g in range(8):
                nc.vector.tensor_scalar(osb[:, 2 * g:2 * (g + 1), :], fm[:, 2 * g:2 * (g + 1), :],
                                        gsumr[:, g:g + 1], None, AL.mult)
            nc.sync.dma_start(p_out[:], osb[:].rearrange("p c t -> p (c t)"))

    nc.compile()
    return nc


# ---------------------------------------------------------------------------
# host prep + launch
# ---------------------------------------------------------------------------

def prepare_in_maps(inputs):
    src = np.asarray(inputs["src"], np.int64)
    dst = np.asarray(inputs["dst"], np.int64)
    assert np.array_equal(dst, np.repeat(np.arange(N, dtype=np.int64), DEG)), "dst structure"
    assert np.array_equal(src // NN, dst // NN), "edges must be graph-local"

    ns = _f32(inputs["ns"]); es = _f32(inputs["es"]); dm = _f32(inputs["dm"])
    mask_fv = _f32(inputs["mask_fv"])

    Wm = [_f32(inputs[f"Wm{k + 1}"]) for k in range(3)]
    Wu = [_f32(inputs[f"Wu{k + 1}"]) for k in range(3)]
    Wx = [_f32(inputs[f"Wx{k + 1}"]) for k in range(3)]
    bx = [_f32(inputs[f"bx{k + 1}"]) for k in range(3)]
    bng = [_f32(inputs[f"bng{k + 1}"]) for k in range(3)]
    bnb = [_f32(inputs[f"bnb{k + 1}"]) for k in range(3)]

    wma_u, wmc_u, wu_u, wx_u, gam_u, bh_u = [], [], [], [], [], []
    for k in range(3):
        cin, cout, Fk, cinp, Fo = CIN[k], COUT[k], NF[k], CINP[k], NFO[k]
        Wma, Wmb, Wmce = Wm[k][:cin], Wm[k][cin:2 * cin], Wm[k][2 * cin:]
        Wmean = Wu[k][cin:2 * cin] + 8.0 * Wu[k][3 * cin:4 * cin]
        Wmax = Wu[k][2 * cin:3 * cin]
        Wstd = Wu[k][4 * cin:]
        Wh = Wu[k][:cin] + Wmb @ (Wmean + Wmax)
        a = np.zeros((128, Fk, cinp), np.float32)
        for ki in range(Fk):
            a[0:CSZ[k][ki], ki, :cin] = Wma[128 * ki:128 * ki + CSZ[k][ki]]
        wma_u.append(_bf(a.reshape(128, -1)))
        c = np.zeros((16, cinp), np.float32)
        c[:, :cin] = Wmce
        wmc_u.append(_bf(c))
        u = np.zeros((128, NSEC[k], cout), np.float32)
        nfull = 3 if k == 1 else Fk
        for si, Wsec in enumerate([Wh, Wmean, Wmax, Wstd]):
            for f in range(nfull):
                u[0:CSZ[k][f], si * nfull + f, :] = Wsec[128 * f:128 * f + CSZ[k][f]]
        if k == 1:
            # packed ragged f=3 chunks: rows = h-d2 | mean | max | std
            for si, Wsec in enumerate([Wh, Wmean, Wmax, Wstd]):
                u[32 * si:32 * (si + 1), 12, :] = Wsec[384:416]
        wu_u.append(_bf(u.reshape(128, -1)))
        if k < 2:
            x = np.zeros((128, Fo, cout), np.float32)
            gcol = np.zeros((128, Fo), np.float32)
            bcol = np.zeros((128, Fo), np.float32)
            bhv = bnb[k] @ Wx[k] + bx[k]
            for mk in range(Fo):
                m = MSZ[k][mk]
                x[0:m, mk, :] = Wx[k][128 * mk:128 * mk + m]
                gcol[0:m, mk] = bng[k][128 * mk:128 * mk + m]
                bcol[0:m, mk] = bhv[128 * mk:128 * mk + m]
            wx_u.append(_bf(x.reshape(128, -1)))
            gam_u.append(_f32(gcol))
            bh_u.append(_f32(bcol))
        else:
            x = np.zeros((128, 2, cout), np.float32)
            x[0:128, 0, :] = Wx[k][0:128]
            x[0:64, 1, :] = Wx[k][128:192]
            x[64, 1, :] = bnb[k] @ Wx[k] + bx[k]       # bias row (pairs with u ones-row)
            wx_u.append(_bf(x.reshape(128, -1)))
            gcol = np.zeros((128, 2), np.float32)
            gcol[0:128, 0] = bng[k][0:128]
            gcol[0:64, 1] = bng[k][128:192]
            gcol[64, 1] = np.sqrt(np.float32(1e-5))    # scale row becomes exactly 1.0
            gam_u.append(_f32(gcol))

    W12 = _f32(inputs["W1"]) @ _f32(inputs["W2"])
    b12v = _f32(inputs["b1"]) @ _f32(inputs["W2"]) + _f32(inputs["b2"])
    w12_u = _bf(W12.reshape(2, 128, 32).transpose(1, 0, 2).reshape(128, -1))
    w3_u = _bf(_f32(inputs["W3"]).reshape(2, 128, 64).transpose(1, 0, 2).reshape(128, -1))
    w4_u = _bf(inputs["W4"])
    b4_u = _f32(np.asarray(inputs["b4"]).reshape(2, 128).T)

    shared = {
        **{f"wma{k}": wma_u[k] for k in range(3)},
        **{f"wmc{k}": wmc_u[k] for k in range(3)},
        **{f"wu{k}": wu_u[k] for k in range(3)},
        **{f"wx{k}": wx_u[k] for k in range(3)},
        **{f"gam{k}": gam_u[k] for k in range(3)},
        **{f"bh{k}": bh_u[k] for k in range(2)},
        "w12": w12_u, "b12": _f32(b12v.reshape(32, 1)),
        "w3": w3_u, "b3": _f32(np.asarray(inputs["b3"]).reshape(64, 1)),
        "w4": w4_u, "b4": b4_u,
    }

    # host-side per-edge es projection C = es @ Wmc (padded), one per layer
    cg_full = []
    for k in range(3):
        cfk = es @ np.frombuffer(wmc_u[k].tobytes(), dtype=ml_dtypes.bfloat16).reshape(
            16, CINP[k]).astype(np.float32)
        cg_full.append(cfk)

    in_maps = []
    ecols = np.arange(2048)
    for c in range(NCORES):
        n0 = NC * c
        srcl = (src[8 * n0:8 * (n0 + NC)] - n0).astype(np.int64)   # 0..2047, natural order
        esl = es[8 * n0:8 * (n0 + NC)]                              # natural order
        oh = np.zeros((128, G, 2, 2048), np.float32)
        mam = np.zeros((G, 256, 256), np.float32)
        for g in range(G):
            sg = srcl[2048 * g:2048 * (g + 1)] - 256 * g            # 0..255
            oh[sg % 128, g, sg // 128, ecols] = 1.0
            dg = ecols // 8
            np.add.at(mam[g], (sg, dg), 0.125)
        cgs = {}
        for k in range(3):
            ck = cg_full[k][8 * n0:8 * (n0 + NC)]          # [16384, cinp]
            cgs[f"cg{k}"] = _bf(ck.reshape(G, 2, 1024, NF[k], 128)
                                .transpose(4, 0, 1, 3, 2).reshape(128, -1))
        in_maps.append({
            "nsT": _bf(ns[n0:n0 + NC].T),
            "dmT": _bf(dm[n0:n0 + NC].T.reshape(2, 128, 2048).transpose(1, 0, 2).reshape(128, -1)),
            "esm": _bf(esl.reshape(NC, DEG, IN_E).mean(1).T),
            **cgs,
            "oh": _bf(oh.reshape(128, -1)),
            "mam": _bf(mam.reshape(G, 2, 128, 256).transpose(2, 0, 1, 3).reshape(128, -1)),
            "mask": _bf(mask_fv[n0:n0 + NC].reshape(16, 128, 192).transpose(1, 0, 2)
                    .reshape(128, -1)),
            **shared,
        })

    return in_maps


def collect_out(res):
    out = np.zeros((B, NN * TP), np.float32)
    for c in range(NCORES):
        oc = res.results[c]["out"].reshape(128, 16, 192).transpose(1, 0, 2).reshape(NC, TP)
        out[G * c:G * (c + 1)] = oc.reshape(G, NN * TP)
    return out


def kernel(**inputs):
    in_maps = prepare_in_maps(inputs)
    nc = _BUILT.get("nc")
    if nc is None:
        nc = build_nc()
        _BUILT["nc"] = nc
    res = run_bass_kernel_spmd(nc, in_maps, list(range(NCORES)))
    _BUILT["last_results"] = res
    return collect_out(res)


# revision 19
# speedup vs baseline: 1.1353x; 1.1353x over previous
"""Trainium2 Bass kernel for Masked_Actor_Net_PNAConv (3x PNAConv + gated masked softmax head).

Sharding: data-parallel by graph across 8 NeuronCores (8 graphs / 2048 nodes /
16384 edges per core). Weights replicated. BatchNorm batch stats are
all-reduced across cores (one [128, 2*Fo] f32 AllReduce per conv layer).

Device-side structure (per core, per layer):
  - h kept feature-major in SBUF arena hT [128, 4, 2048] bf16
  - A = h @ Wm_src computed node-major on PE -> a_sb (SBUF only, no DRAM trip)
  - per-edge messages materialized as PE matmuls: for each 512-edge tile,
    msg^T[f, e] accumulates 3 matmuls in PSUM: one-hot(src lo chunk) @ A_lo +
    one-hot(src hi chunk) @ A_hi + Wmc^T @ es^T  (no DMA gather, no gpsimd)
  - edge columns ordered j-innermost (node-major, 8 edges contiguous):
    max agg = vector pool_max over window 8 straight from PSUM;
    sumsq agg = scalar Square eviction + vector pool_avg (gives E[msg^2])
  - mean agg via mean-adjacency (counts/8) matmuls + es-mean term on PE
  - U matmuls run full-width (512-node tiles across graphs); BatchNorm folded
    into the mixing Linear after the stats AllReduce (as before)
"""
import sys
sys.path.insert(0, '/opt/trn_rl_repo')
import contextlib
import numpy as np
import ml_dtypes

import concourse.bacc as bacc
import concourse.mybir as mybir
import concourse.bass_isa as bass_isa
from concourse import tile
from concourse.bass_utils import run_bass_kernel_spmd

BF = mybir.dt.bfloat16
F32 = mybir.dt.float32
U8 = mybir.dt.uint8
AL = mybir.AluOpType
AF = mybir.ActivationFunctionType
AX = mybir.AxisListType

B, NN, DEG = 64, 256, 8
N, E = B * NN, B * NN * DEG
IN_N, IN_E = 128, 16
TP = 192
H1 = 384
NCORES = 8
G = B // NCORES        # 8 graphs per core
NC = G * NN            # 2048 nodes per core
EC = NC * DEG          # 16384 edges per core

CIN = [IN_N, H1 + 32, H1]                 # 128, 416, 384
COUT = [H1, H1, TP]                       # 384, 384, 192
NF = [(c + 127) // 128 for c in CIN]      # 1, 4, 3
CINP = [128 * f for f in NF]              # 128, 512, 384
NFO = [(c + 127) // 128 for c in COUT]    # 3, 3, 2
CSZ = [[min(128, CIN[k] - 128 * i) for i in range(NF[k])] for k in range(3)]
MSZ = [[min(128, COUT[k] - 128 * i) for i in range(NFO[k])] for k in range(3)]
NSEC = [4, 13, 12]  # U weight sections per layer (L1: 4x3 full + 1 packed)

_BUILT = {}


def _bf(x):
    return np.ascontiguousarray(np.asarray(x, np.float32).astype(ml_dtypes.bfloat16))


def _f32(x):
    return np.ascontiguousarray(np.asarray(x, np.float32))


# ---------------------------------------------------------------------------
# device kernel (SPMD, identical program on all 8 cores)
# ---------------------------------------------------------------------------

def build_nc():
    nc = bacc.Bacc(None, target_bir_lowering=False, debug=True, dynamic_dma_scratch_size=32768)

    def par(name, shape, dt, out=False):
        return nc.declare_dram_parameter(name, list(shape), dt, isOutput=out)

    p_nsT = par("nsT", [128, 2048], BF)
    p_dmT = par("dmT", [128, 2 * 2048], BF)
    p_esm = par("esm", [16, 2048], BF)
    p_cg = [par(f"cg{k}", [128, G * 2 * NF[k] * 1024], BF) for k in range(3)]
    p_oh = par("oh", [128, G * 2 * 2048], BF)
    p_mam = par("mam", [128, G * 2 * 256], BF)
    p_mask = par("mask", [128, 16 * 192], BF)
    p_wma = [par(f"wma{k}", [128, NF[k] * CINP[k]], BF) for k in range(3)]
    p_wmc = [par(f"wmc{k}", [16, CINP[k]], BF) for k in range(3)]
    p_wu = [par(f"wu{k}", [128, NSEC[k] * COUT[k]], BF) for k in range(3)]
    p_wx = [par(f"wx{k}", [128, NFO[k] * COUT[k]], BF) for k in range(3)]
    p_gam = [par(f"gam{k}", [128, NFO[k]], F32) for k in range(3)]
    p_bh = [par(f"bh{k}", [128, NFO[k]], F32) for k in range(2)]
    p_w12 = par("w12", [128, 2 * 32], BF)
    p_b12 = par("b12", [32, 1], F32)
    p_w3 = par("w3", [128, 2 * 64], BF)
    p_b3 = par("b3", [64, 1], F32)
    p_w4 = par("w4", [64, 256], BF)
    p_b4 = par("b4", [128, 2], F32)
    p_out = par("out", [128, 16 * 192], F32, out=True)

    with tile.TileContext(nc) as tc:
        with contextlib.ExitStack() as ctx:
            stat = ctx.enter_context(tc.tile_pool(name="stat", bufs=1))
            ohp = ctx.enter_context(tc.tile_pool(name="ohp", bufs=2))    # one-hot per graph
            cgp = ctx.enter_context(tc.tile_pool(name="cgp", bufs=2))    # host C slices
            msgp = ctx.enter_context(tc.tile_pool(name="msgp", bufs=2))  # evicted msg tiles
            sqp = ctx.enter_context(tc.tile_pool(name="sqp", bufs=2))    # square scratch
            wupool = ctx.enter_context(tc.tile_pool(name="wupool", bufs=1))
            dpool = ctx.enter_context(tc.tile_pool(name="dpool", bufs=1, space="DRAM"))
            psM = ctx.enter_context(tc.tile_pool(name="psM", bufs=2, space="PSUM"))
            psU = ctx.enter_context(tc.tile_pool(name="psU", bufs=2, space="PSUM"))
            psW = ctx.enter_context(tc.tile_pool(name="psW", bufs=2, space="PSUM"))

            def load(shape, dt, src, tag, pool=None):
                t = (pool or stat).tile(list(shape), dt, tag=tag, name=tag)
                nc.sync.dma_start(t[:], src[:])
                return t

            hT = stat.tile([128, 4, 2048], BF, tag="hT", name="hT")
            nc.sync.dma_start(hT[:, 0, :], p_nsT[:])
            uT = stat.tile([128, 3, 2048], BF, tag="uT")
            pmax = stat.tile([128, 4, 2048], BF, tag="pmax")
            pmean = stat.tile([128, 4, 2048], BF, tag="pmean")
            pstd = stat.tile([128, 4, 2048], BF, tag="pstd")
            a_sb = stat.tile([128, 16, 512], BF, tag="a_sb")
            qmean = stat.tile([128, 4, 256], F32, tag="qmean")
            vtmp = stat.tile([128, 4, 256], F32, tag="vtmp")
            esm = load([16, 2048], BF, p_esm, "esm")
            mam = load([128, G, 2, 256], BF, p_mam, "mam")
            packX = None  # allocated at use; shares the dmT slot

            wmc = [load([16, CINP[k]], BF, p_wmc[k], f"wmc{k}s") for k in range(3)]
            wx = [load([128, NFO[k], COUT[k]], BF, p_wx[k], f"wx{k}s") for k in range(3)]
            gam = [load([128, NFO[k]], F32, p_gam[k], f"gam{k}s") for k in range(3)]
            bh = [load([128, NFO[k]], F32, p_bh[k], f"bh{k}s") for k in range(2)]
            w12 = load([128, 2, 32], BF, p_w12, "w12")
            b12 = load([32, 1], F32, p_b12, "b12")
            w3 = load([128, 2, 64], BF, p_w3, "w3")
            b3 = load([64, 1], F32, p_b3, "b3")
            w4 = load([64, 256], BF, p_w4, "w4")
            b4 = load([128, 2], F32, p_b4, "b4")
            wxs = stat.tile([128, 3, 384], BF, tag="wxs")
            cc_in = [dpool.tile([128, 2 * NFO[k]], F32, tag=f"ccin{k}", name=f"ccin{k}") for k in range(3)]
            cc_out = [dpool.tile([128, 2 * NFO[k]], F32, tag=f"ccout{k}", name=f"ccout{k}") for k in range(3)]

            # ---- d2 = dm @ (W1 @ W2) + b12 -> hT chunk 3 rows 0:32 -----------
            dmT = stat.tile([128, 2, 2048], BF, tag="packX")  # slot shared with packX
            nc.sync.dma_start(dmT[:].rearrange("p c n -> p (c n)"), p_dmT[:])
            for n4 in range(4):
                ps = psW.tile([128, 512], F32, tag="psW")
                for kc in range(2):
                    nc.tensor.matmul(ps[0:32, :], w12[:, kc, :],
                                     dmT[:, kc, 512 * n4:512 * (n4 + 1)],
                                     start=(kc == 0), stop=(kc == 1))
                nc.scalar.activation(hT[0:32, 3, 512 * n4:512 * (n4 + 1)], ps[0:32, :],
                                     AF.Identity, bias=b12[:, 0:1])

            h3 = stat.tile([128, 16, 192], BF, tag="h3")
            c30 = stat.tile([128, 1], F32, tag="c30")
            nc.vector.memset(c30[:], 1e-30)
            c5 = stat.tile([128, 1], F32, tag="c5")
            nc.vector.memset(c5[:], 1e-5)
            uaccS = stat.tile([128, 3, 4], F32, tag="uaccS")
            uaccQ = stat.tile([128, 3, 4], F32, tag="uaccQ")

            # ---- conv layers -------------------------------------------------
            for k in range(3):
                F = NF[k]
                cinp, cout, Fo = CINP[k], COUT[k], NFO[k]
                csz, msz = CSZ[k], MSZ[k]

                wu_k = load([128, NSEC[k], cout], BF, p_wu[k], "wu_k", pool=wupool)
                wma_k = load([128, F, cinp], BF, p_wma[k], "wma_k", pool=wupool)
                nc.vector.memset(uaccS[:], 0.0)
                nc.vector.memset(uaccQ[:], 0.0)
                if k == 2:
                    nc.vector.memset(uT[64:128, 1, :], 0.0)

                # A = h @ Wma (node-major) -> a_sb
                for t in range(16):
                    ps = psU.tile([128, 512], F32, tag="psU")
                    for ki in range(F):
                        nc.tensor.matmul(ps[:, 0:cinp],
                                         hT[0:csz[ki], ki, 128 * t:128 * (t + 1)],
                                         wma_k[0:csz[ki], ki, :],
                                         start=(ki == 0), stop=(ki == F - 1))
                    nc.scalar.activation(a_sb[:, t, 0:cinp], ps[:, 0:cinp], AF.Copy, bias=0.0)

                # aggregation phase, per graph
                for g in range(G):
                    ohg = ohp.tile([128, 2, 2048], BF, tag="ohg")
                    nc.sync.dma_start(ohg[:].rearrange("p c e -> p (c e)"),
                                      p_oh[:, g * 4096:(g + 1) * 4096])

                    for f in range(F):
                        # mean aggregation: A^T @ madj/8 + Wmc^T @ es_mean
                        pm = psU.tile([128, 512], F32, tag="psU")
                        nc.tensor.matmul(pm[:, 0:256], a_sb[:, 2 * g, 128 * f:128 * (f + 1)],
                                         mam[:, g, 0, :], start=True, stop=False)
                        nc.tensor.matmul(pm[:, 0:256], a_sb[:, 2 * g + 1, 128 * f:128 * (f + 1)],
                                         mam[:, g, 1, :], start=False, stop=False)
                        nc.tensor.matmul(pm[:, 0:256], wmc[k][0:16, 128 * f:128 * (f + 1)],
                                         esm[0:16, 256 * g:256 * (g + 1)],
                                         start=False, stop=True)
                        nc.scalar.activation(pmean[:, f, 256 * g:256 * (g + 1)], pm[:, 0:256],
                                             AF.Copy, bias=0.0)

                    # messages: PE does the two one-hot gathers only; the es
                    # contribution C = es @ Wmc is host-computed and added on
                    # the (otherwise idle) gpsimd after a bf16 eviction.
                    for h2 in range(2):
                        cgh = cgp.tile([128, F, 1024], BF, tag="cgh")
                        nc.sync.dma_start(
                            cgh[:].rearrange("p f e -> p (f e)"),
                            p_cg[k][:, (g * 2 + h2) * F * 1024:(g * 2 + h2 + 1) * F * 1024])
                        for f in range(F):
                            pmsg = psM.tile([128, 2, 512], F32, tag="psM")
                            for e2 in range(2):
                                c0 = 1024 * h2 + 512 * e2
                                nc.tensor.matmul(pmsg[:, e2, :],
                                                 a_sb[:, 2 * g, 128 * f:128 * (f + 1)],
                                                 ohg[:, 0, c0:c0 + 512], start=True, stop=False)
                                nc.tensor.matmul(pmsg[:, e2, :],
                                                 a_sb[:, 2 * g + 1, 128 * f:128 * (f + 1)],
                                                 ohg[:, 1, c0:c0 + 512], start=False, stop=True)
                            msgb = msgp.tile([128, 1024], BF, tag="msgb")
                            nc.scalar.activation(msgb[:], pmsg[:].rearrange("p a e -> p (a e)"),
                                                 AF.Copy, bias=0.0)
                            nc.vector.tensor_tensor(msgb[:], msgb[:], cgh[:, f, :], AL.add)
                            # max over the 8 edges of each node (j innermost)
                            nc.vector.tensor_reduce(
                                pmax[:, f, 256 * g + 128 * h2:256 * g + 128 * (h2 + 1)],
                                msgb[:].rearrange("p (n j) -> p n j", j=8), AX.X, AL.max)
                            sq = sqp.tile([128, 1024], BF, tag="sq")
                            nc.scalar.activation(sq[:], msgb[:], AF.Square)
                            nc.vector.tensor_reduce(
                                qmean[:, f, 128 * h2:128 * (h2 + 1)],
                                sq[:].rearrange("p (n j) -> p n j", j=8), AX.X, AL.add)

                    # std = sqrt(relu(Q/8 - mean^2) + 1e-30)
                    nc.scalar.activation(qmean[:, 0:F, :], qmean[:, 0:F, :],
                                         AF.Copy, bias=0.0, scale=0.125)
                    nc.vector.tensor_tensor(vtmp[:, 0:F, :],
                                            pmean[:, 0:F, 256 * g:256 * (g + 1)],
                                            pmean[:, 0:F, 256 * g:256 * (g + 1)], AL.mult)
                    nc.vector.tensor_tensor(qmean[:, 0:F, :], qmean[:, 0:F, :],
                                            vtmp[:, 0:F, :], AL.subtract)
                    nc.scalar.activation(qmean[:, 0:F, :], qmean[:, 0:F, :], AF.Relu)
                    nc.scalar.activation(pstd[:, 0:F, 256 * g:256 * (g + 1)],
                                         qmean[:, 0:F, :], AF.Sqrt, bias=c30[:, 0:1])

                # U matmuls: X = [h | mean | max | std], folded weights, 512-wide.
                # For L1 the four ragged 32-row f=3 chunks are packed into one
                # K=128 section (rows: h-d2 | mean | max | std), built via DMA.
                xs = [None, pmean, pmax, pstd]
                if k == 1:
                    packX = stat.tile([128, 2048], BF, tag="packX")
                    nc.sync.dma_start(packX[0:32, :], hT[0:32, 3, :])
                    nc.sync.dma_start(packX[32:64, :], pmean[0:32, 3, :])
                    nc.sync.dma_start(packX[64:96, :], pmax[0:32, 3, :])
                    nc.sync.dma_start(packX[96:128, :], pstd[0:32, 3, :])
                    nfull = 3
                else:
                    nfull = F
                nsec = 4 * nfull + (1 if k == 1 else 0)
                for mo in range(Fo):
                    mi = msz[mo]
                    for n4 in range(4):
                        ps = psU.tile([128, 512], F32, tag="psU")
                        i = 0
                        for sect in range(4):
                            for f in range(nfull):
                                if sect == 0:
                                    rhs = hT[0:csz[f], f, 512 * n4:512 * (n4 + 1)]
                                else:
                                    rhs = xs[sect][0:csz[f], f, 512 * n4:512 * (n4 + 1)]
                                nc.tensor.matmul(
                                    ps[0:mi, :],
                                    wu_k[0:csz[f], sect * nfull + f, 128 * mo:128 * mo + mi],
                                    rhs, start=(i == 0), stop=(i == nsec - 1))
                                i += 1
                        if k == 1:
                            nc.tensor.matmul(
                                ps[0:mi, :],
                                wu_k[:, 4 * nfull, 128 * mo:128 * mo + mi],
                                packX[:, 512 * n4:512 * (n4 + 1)],
                                start=False, stop=True)
                            i += 1
                        nc.scalar.activation(uT[0:mi, mo, 512 * n4:512 * (n4 + 1)], ps[0:mi, :],
                                             AF.Copy, bias=0.0,
                                             accum_out=uaccS[0:mi, mo, n4:n4 + 1])
                        usq = sqp.tile([128, 1024], BF, tag="sq")
                        nc.scalar.activation(usq[0:mi, 0:512], uT[0:mi, mo, 512 * n4:512 * (n4 + 1)],
                                             AF.Square, accum_out=uaccQ[0:mi, mo, n4:n4 + 1])

                # ---- BN stats all-reduce, fold into mixing ----
                ccs = stat.tile([128, 6], F32, tag="ccs")
                nc.vector.tensor_reduce(ccs[:, 0:Fo], uaccS[:, 0:Fo, :], AX.X, AL.add)
                nc.vector.tensor_reduce(ccs[:, Fo:2 * Fo], uaccQ[:, 0:Fo, :], AX.X, AL.add)
                nc.sync.dma_start(cc_in[k][:], ccs[:, 0:2 * Fo])
                import os as _os
                _rg = [[i] for i in range(NCORES)] if _os.environ.get("KERN_NO_CC") else [list(range(NCORES))]
                nc.gpsimd.collective_compute(
                    "AllReduce", AL.add, replica_groups=_rg,
                    ins=[cc_in[k].opt()], outs=[cc_out[k].opt()])
                ccr = stat.tile([128, 6], F32, tag="ccr")
                nc.sync.dma_start(ccr[:, 0:2 * Fo], cc_out[k][:])
                mu = stat.tile([128, 3], F32, tag="mu")
                sc = stat.tile([128, 3], F32, tag="sc")
                mu2 = stat.tile([128, 3], F32, tag="mu2")
                nc.scalar.activation(mu[:, 0:Fo], ccr[:, 0:Fo], AF.Copy, bias=0.0, scale=1.0 / N)
                nc.scalar.activation(sc[:, 0:Fo], ccr[:, Fo:2 * Fo], AF.Copy, bias=0.0, scale=1.0 / N)
                nc.vector.tensor_tensor(mu2[:, 0:Fo], mu[:, 0:Fo], mu[:, 0:Fo], AL.mult)
                nc.vector.tensor_tensor(sc[:, 0:Fo], sc[:, 0:Fo], mu2[:, 0:Fo], AL.subtract)
                nc.scalar.activation(sc[:, 0:Fo], sc[:, 0:Fo], AF.Sqrt, bias=c5[:, 0:1])
                nc.vector.reciprocal(sc[:, 0:Fo], sc[:, 0:Fo])
                nc.vector.tensor_tensor(sc[:, 0:Fo], sc[:, 0:Fo], gam[k][:, 0:Fo], AL.mult)
                for mo in range(Fo):
                    mi = msz[mo]
                    nc.vector.tensor_scalar(uT[0:mi, mo, :], uT[0:mi, mo, :],
                                            mu[0:mi, mo:mo + 1], None, AL.subtract)
                    nc.vector.tensor_scalar(wxs[:, mo, 0:cout], wx[k][:, mo, 0:cout],
                                            sc[:, mo:mo + 1], None, AL.mult)
                if k == 2:
                    nc.vector.memset(uT[64:65, 1, :], 1.0)
                # mixing matmul (+ BN shift via bias / ones-row), relu(leaky) = relu
                if k < 2:
                    for mo in range(Fo):
                        for n4 in range(4):
                            ps = psW.tile([128, 512], F32, tag="psW")
                            for mk in range(Fo):
                                nc.tensor.matmul(ps[:, :],
                                                 wxs[0:msz[mk], mk, 128 * mo:128 * (mo + 1)],
                                                 uT[0:msz[mk], mk, 512 * n4:512 * (n4 + 1)],
                                                 start=(mk == 0), stop=(mk == Fo - 1))
                            nc.scalar.activation(hT[:, mo, 512 * n4:512 * (n4 + 1)], ps[:, :],
                                                 AF.Relu, bias=bh[k][:, mo:mo + 1])
                else:
                    for t in range(16):
                        ps = psW.tile([128, 512], F32, tag="psW")
                        nc.tensor.matmul(ps[:, 0:192], uT[0:128, 0, 128 * t:128 * (t + 1)],
                                         wxs[0:128, 0, 0:192], start=True, stop=False)
                        nc.tensor.matmul(ps[:, 0:192], uT[0:65, 1, 128 * t:128 * (t + 1)],
                                         wxs[0:65, 1, 0:192], start=False, stop=True)
                        nc.scalar.activation(h3[:, t, :], ps[:, 0:192], AF.Lrelu, alpha=0.01)

            # ---- head --------------------------------------------------------
            mask = load([128, 16, 192], BF, p_mask, "wu_k", pool=wupool)  # wu slot is dead now
            nmx = stat.tile([128, 16], BF, tag="nmx")
            nc.vector.tensor_reduce(nmx[:], h3[:], AX.X, AL.max)
            ps3 = psW.tile([128, 512], F32, tag="psW")
            nc.tensor.matmul(ps3[0:64, 0:8], w3[:, 0, :], nmx[:, 0::2], start=True, stop=False)
            nc.tensor.matmul(ps3[0:64, 0:8], w3[:, 1, :], nmx[:, 1::2], start=False, stop=True)
            r3 = stat.tile([64, 8], BF, tag="r3")
            nc.scalar.activation(r3[:], ps3[0:64, 0:8], AF.Relu, bias=b3[:, 0:1])
            gn = stat.tile([128, 16], F32, tag="gn")
            for half in range(2):
                ps4 = psW.tile([128, 512], F32, tag="psW")
                nc.tensor.matmul(ps4[:, 0:8], w4[0:64, 128 * half:128 * (half + 1)], r3[:],
                                 start=True, stop=True)
                nc.scalar.activation(gn[:, half::2], ps4[:, 0:8], AF.Sigmoid,
                                     bias=b4[:, half:half + 1])
            feat = stat.tile([128, 16, 192], F32, tag="pmax")   # reuse agg slots at head time
            for c in range(16):
                nc.vector.tensor_scalar(feat[:, c, :], h3[:, c, :], gn[:, c:c + 1], None, AL.mult)
            fm = stat.tile([128, 16, 192], F32, tag="pmean")
            # fm = (feat + 1e5) * mask01: masked lanes -> 0, ~1e5 below valid
            nc.vector.scalar_tensor_tensor(fm[:], feat[:], 1e5, mask[:],
                                           AL.add, AL.mult)
            gmax = stat.tile([128, 8], F32, tag="gmax")
            gmaxr = stat.tile([128, 8], F32, tag="gmaxr")
            nc.vector.tensor_reduce(gmax[:], fm[:].rearrange("p (g x) t -> p g (x t)", g=8), AX.X, AL.max)
            nc.gpsimd.partition_all_reduce(gmaxr[:], gmax[:], 128, bass_isa.ReduceOp.max)
            for g in range(8):
                nc.vector.tensor_scalar(fm[:, 2 * g:2 * (g + 1), :], fm[:, 2 * g:2 * (g + 1), :],
                                        gmaxr[:, g:g + 1], None, AL.subtract)
            nc.scalar.activation(fm[:], fm[:], AF.Exp)
            gsum = stat.tile([128, 8], F32, tag="gsum")
            gsumr = stat.tile([128, 8], F32, tag="gsumr")
            nc.vector.tensor_reduce(gsum[:], fm[:].rearrange("p (g x) t -> p g (x t)", g=8), AX.X, AL.add)
            nc.gpsimd.partition_all_reduce(gsumr[:], gsum[:], 128, bass_isa.ReduceOp.add)
            nc.vector.reciprocal(gsumr[:], gsumr[:])
            osb = stat.tile([128, 16, 192], F32, tag="pstd")
            for g in range(8):
                nc.vector.tensor_scalar(osb[:, 2 * g:2 * (g + 1), :], fm[:, 2 * g:2 * (g + 1), :],
                                        gsumr[:, g:g + 1], None, AL.mult)
            nc.sync.dma_start(p_out[:], osb[:].rearrange("p c t -> p (c t)"))

    nc.compile()
    return nc


# ---------------------------------------------------------------------------
# host prep + launch
# ---------------------------------------------------------------------------

def prepare_in_maps(inputs):
    src = np.asarray(inputs["src"], np.int64)
    dst = np.asarray(inputs["dst"], np.int64)
    assert np.array_equal(dst, np.repeat(np.arange(N, dtype=np.int64), DEG)), "dst structure"
    assert np.array_equal(src // NN, dst // NN), "edges must be graph-local"

    ns = _f32(inputs["ns"]); es = _f32(inputs["es"]); dm = _f32(inputs["dm"])
    mask_fv = _f32(inputs["mask_fv"])

    Wm = [_f32(inputs[f"Wm{k + 1}"]) for k in range(3)]
    Wu = [_f32(inputs[f"Wu{k + 1}"]) for k in range(3)]
    Wx = [_f32(inputs[f"Wx{k + 1}"]) for k in range(3)]
    bx = [_f32(inputs[f"bx{k + 1}"]) for k in range(3)]
    bng = [_f32(inputs[f"bng{k + 1}"]) for k in range(3)]
    bnb = [_f32(inputs[f"bnb{k + 1}"]) for k in range(3)]

    wma_u, wmc_u, wu_u, wx_u, gam_u, bh_u = [], [], [], [], [], []
    for k in range(3):
        cin, cout, Fk, cinp, Fo = CIN[k], COUT[k], NF[k], CINP[k], NFO[k]
        Wma, Wmb, Wmce = Wm[k][:cin], Wm[k][cin:2 * cin], Wm[k][2 * cin:]
        Wmean = Wu[k][cin:2 * cin] + 8.0 * Wu[k][3 * cin:4 * cin]
        Wmax = Wu[k][2 * cin:3 * cin]
        Wstd = Wu[k][4 * cin:]
        Wh = Wu[k][:cin] + Wmb @ (Wmean + Wmax)
        a = np.zeros((128, Fk, cinp), np.float32)
        for ki in range(Fk):
            a[0:CSZ[k][ki], ki, :cin] = Wma[128 * ki:128 * ki + CSZ[k][ki]]
        wma_u.append(_bf(a.reshape(128, -1)))
        c = np.zeros((16, cinp), np.float32)
        c[:, :cin] = Wmce
        wmc_u.append(_bf(c))
        u = np.zeros((128, NSEC[k], cout), np.float32)
        nfull = 3 if k == 1 else Fk
        for si, Wsec in enumerate([Wh, Wmean, Wmax, Wstd]):
            for f in range(nfull):
                u[0:CSZ[k][f], si * nfull + f, :] = Wsec[128 * f:128 * f + CSZ[k][f]]
        if k == 1:
            # packed ragged f=3 chunks: rows = h-d2 | mean | max | std
            for si, Wsec in enumerate([Wh, Wmean, Wmax, Wstd]):
                u[32 * si:32 * (si + 1), 12, :] = Wsec[384:416]
        wu_u.append(_bf(u.reshape(128, -1)))
        if k < 2:
            x = np.zeros((128, Fo, cout), np.float32)
            gcol = np.zeros((128, Fo), np.float32)
            bcol = np.zeros((128, Fo), np.float32)
            bhv = bnb[k] @ Wx[k] + bx[k]
            for mk in range(Fo):
                m = MSZ[k][mk]
                x[0:m, mk, :] = Wx[k][128 * mk:128 * mk + m]
                gcol[0:m, mk] = bng[k][128 * mk:128 * mk + m]
                bcol[0:m, mk] = bhv[128 * mk:128 * mk + m]
            wx_u.append(_bf(x.reshape(128, -1)))
            gam_u.append(_f32(gcol))
            bh_u.append(_f32(bcol))
        else:
            x = np.zeros((128, 2, cout), np.float32)
            x[0:128, 0, :] = Wx[k][0:128]
            x[0:64, 1, :] = Wx[k][128:192]
            x[64, 1, :] = bnb[k] @ Wx[k] + bx[k]       # bias row (pairs with u ones-row)
            wx_u.append(_bf(x.reshape(128, -1)))
            gcol = np.zeros((128, 2), np.float32)
            gcol[0:128, 0] = bng[k][0:128]
            gcol[0:64, 1] = bng[k][128:192]
            gcol[64, 1] = np.sqrt(np.float32(1e-5))    # scale row becomes exactly 1.0
            gam_u.append(_f32(gcol))

    W12 = _f32(inputs["W1"]) @ _f32(inputs["W2"])
    b12v = _f32(inputs["b1"]) @ _f32(inputs["W2"]) + _f32(inputs["b2"])
    w12_u = _bf(W12.reshape(2, 128, 32).transpose(1, 0, 2).reshape(128, -1))
    w3_u = _bf(_f32(inputs["W3"]).reshape(2, 128, 64).transpose(1, 0, 2).reshape(128, -1))
    w4_u = _bf(inputs["W4"])
    b4_u = _f32(np.asarray(inputs["b4"]).reshape(2, 128).T)

    shared = {
        **{f"wma{k}": wma_u[k] for k in range(3)},
        **{f"wmc{k}": wmc_u[k] for k in range(3)},
        **{f"wu{k}": wu_u[k] for k in range(3)},
        **{f"wx{k}": wx_u[k] for k in range(3)},
        **{f"gam{k}": gam_u[k] for k in range(3)},
        **{f"bh{k}": bh_u[k] for k in range(2)},
        "w12": w12_u, "b12": _f32(b12v.reshape(32, 1)),
        "w3": w3_u, "b3": _f32(np.asarray(inputs["b3"]).reshape(64, 1)),
        "w4": w4_u, "b4": b4_u,
    }

    # host-side per-edge es projection C = es @ Wmc (padded), one per layer
    cg_full = []
    for k in range(3):
        cfk = es @ np.frombuffer(wmc_u[k].tobytes(), dtype=ml_dtypes.bfloat16).reshape(
            16, CINP[k]).astype(np.float32)
        cg_full.append(cfk)

    in_maps = []
    ecols = np.arange(2048)
    for c in range(NCORES):
        n0 = NC * c
        srcl = (src[8 * n0:8 * (n0 + NC)] - n0).astype(np.int64)   # 0..2047, natural order
        esl = es[8 * n0:8 * (n0 + NC)]                              # natural order
        oh = np.zeros((128, G, 2, 2048), np.float32)
        mam = np.zeros((G, 256, 256), np.float32)
        for g in range(G):
            sg = srcl[2048 * g:2048 * (g + 1)] - 256 * g            # 0..255
            oh[sg % 128, g, sg // 128, ecols] = 1.0
            dg = ecols // 8
            np.add.at(mam[g], (sg, dg), 0.125)
        cgs = {}
        for k in range(3):
            ck = cg_full[k][8 * n0:8 * (n0 + NC)]          # [16384, cinp]
            cgs[f"cg{k}"] = _bf(ck.reshape(G, 2, 1024, NF[k], 128)
                                .transpose(4, 0, 1, 3, 2).reshape(128, -1))
        in_maps.append({
            "nsT": _bf(ns[n0:n0 + NC].T),
            "dmT": _bf(dm[n0:n0 + NC].T.reshape(2, 128, 2048).transpose(1, 0, 2).reshape(128, -1)),
            "esm": _bf(esl.reshape(NC, DEG, IN_E).mean(1).T),
            **cgs,
            "oh": _bf(oh.reshape(128, -1)),
            "mam": _bf(mam.reshape(G, 2, 128, 256).transpose(2, 0, 1, 3).reshape(128, -1)),
            "mask": _bf(mask_fv[n0:n0 + NC].reshape(16, 128, 192).transpose(1, 0, 2)
                    .reshape(128, -1)),
            **shared,
        })

    return in_maps


def collect_out(res):
    out = np.zeros((B, NN * TP), np.float32)
    for c in range(NCORES):
        oc = res.results[c]["out"].reshape(128, 16, 192).transpose(1, 0, 2).reshape(NC, TP)
        out[G * c:G * (c + 1)] = oc.reshape(G, NN * TP)
    return out


def kernel(**inputs):
    in_maps = prepare_in_maps(inputs)
    nc = _BUILT.get("nc")
    if nc is None:
        nc = build_nc()
        _BUILT["nc"] = nc
    res = run_bass_kernel_spmd(nc, in_maps, list(range(NCORES)))
    _BUILT["last_results"] = res
    return collect_out(res)
